# revision 15
# baseline (speedup 1.0000x reference)
"""Trainium2 Bass kernel for BertLinearSelfAttention (linear attention).

Reference computation (per batch b, head h):
    q,k,v = X @ W{q,k,v} + b{q,k,v}            # [S, D] -> heads of 64
    qf, kf = elu(q)+1, elu(k)+1                # = min(exp(x),1) + max(x,0)
    kv[d,e]  = sum_s kf[s,d] v[s,e]            # [64, 64]
    ksum[d]  = sum_s kf[s,d]
    out[s,e] = (sum_d qf[s,d] kv[d,e]) / (sum_d qf[s,d] ksum[d])

Sharding: 8 cores = (4 batches) x (2 head-groups of 8 heads / 512 proj cols).

Everything runs in bf16 (f32 PSUM accumulation): bf16 is the fastest
matmul dtype at ANY output size (fp32r drops to 1/4 rate below 256
moving rows), stationary loads are half-cost, input DMA halves, and DVE
elementwise ops on all-SBUF bf16 operands run at up to 4x.  Measured
end-to-end L2 error of the full bf16 pipeline is ~2e-3 (tolerance 2e-2).

Round 1 (k/v) per 512-token chunk: X^T tiles stationary, W moving
(N=512).  Feature map kf = exp(min(x,0)) + max(x,0):
    x  = k + bk          (DVE, psum + broadcast-bias tile)
    rel= Relu(-x)        (ACT, scale=-1)
    m  = Exp(-rel)       (ACT, scale=-1)  == min(exp(x),1)
    r  = max(x,0)        (DVE 4x)
    kf = m + r           (DVE 4x, bf16)
kv/ksum matmuls run with heads PAIRED: moving layout [vA|vB|ones|pad]
(N=130) against a [128tok,128] kf stationary -> block-diag kv plus a
shared ksum column, accumulated per chunk into SBUF f32.

Round 2 (q/out): q^T projection (Wq stationary, X^T moving, N=512
tokens).  The numerator is computed TRANSPOSED: the bf16 block-diag kv
is the stationary operand, qf^T the moving one -> one 512-row matmul
per head-pair.  The per-head denominator is produced directly in
broadcast form by a second matmul whose stationary is the block-diag
ksum replication [d,e]=ksum[d]*same_head(d,e); out = num/den is then a
single DVE divide per column tile.  Output is stored transposed
([CG, S]) and transposed back on the host.

bv is never added to v on-device: kv_true = kv_nobias + ksum (x) bv is
applied once to the tiny kv matrix (bv enters kv linearly).

The PE is warmed up with dummy matmuls during the initial DMA fill (the
clock p-state needs ~3us of continuous work to reach 2.4 GHz), and all
cross-engine consumers are emitted one chunk behind their producers so
the PE never waits on the ACT/DVE feature chain.
"""

import os
import sys

import numpy as np

_REPO = "/opt/trn_rl_repo"
if os.path.isdir(_REPO) and _REPO not in sys.path:
    sys.path.insert(0, _REPO)

B, S, D, H, HD = 4, 4096, 1024, 16, 64
NCORES = 8
CG = 512            # projection columns per core (8 heads)
NH = CG // HD       # 8 heads per core
NPAIR = NH // 2     # 4 head pairs (= column tiles of 128)
PW = 130            # pair width in kv psum: [vA(64) | vB(64) | ones | pad]
CHUNK = 512         # tokens per chunk
NSUB = CHUNK // 128     # 4 token sub-tiles per chunk
NCHUNK = S // CHUNK     # 8 chunks
NKT = D // 128          # 8 contraction tiles
NCT = CG // 128         # 4 column tiles (= head pairs)
P = 128

_CACHED_NC = None


def _build():
    import concourse.tile as tile
    from concourse import bacc, mybir
    from contextlib import ExitStack

    F32 = mybir.dt.float32
    BF16 = mybir.dt.bfloat16
    Alu = mybir.AluOpType
    Act = mybir.ActivationFunctionType

    nc = bacc.Bacc("TRN2", target_bir_lowering=False, debug=False,
                   num_devices=NCORES)

    xt_d = nc.dram_tensor("xt", [D, S], BF16, kind="ExternalInput").ap()
    wk_d = nc.dram_tensor("wk", [D, CG], BF16, kind="ExternalInput").ap()
    wv_d = nc.dram_tensor("wv", [D, CG], BF16, kind="ExternalInput").ap()
    wq_d = nc.dram_tensor("wq", [D, CG], BF16, kind="ExternalInput").ap()
    bq_d = nc.dram_tensor("bq", [CG], F32, kind="ExternalInput").ap()
    bkr_d = nc.dram_tensor("bkr", [P, CG], BF16, kind="ExternalInput").ap()
    bvbd_d = nc.dram_tensor("bvbd", [P, CG], F32, kind="ExternalInput").ap()
    out_d = nc.dram_tensor("out", [CG, S], F32, kind="ExternalOutput").ap()

    with tile.TileContext(nc) as tc:
        with ExitStack() as ctx:
            const = ctx.enter_context(tc.tile_pool(name="const", bufs=1))
            wpool = ctx.enter_context(tc.tile_pool(name="wpool", bufs=1))
            xtpool = ctx.enter_context(tc.tile_pool(name="xtpool", bufs=16))
            kfpool = ctx.enter_context(tc.tile_pool(name="kfpool", bufs=9))
            vppool = ctx.enter_context(tc.tile_pool(name="vppool", bufs=9))
            qftpool = ctx.enter_context(tc.tile_pool(name="qftpool", bufs=9))
            tmp = ctx.enter_context(tc.tile_pool(name="tmp", bufs=12))
            stg = ctx.enter_context(tc.tile_pool(name="stg", bufs=2))
            outpool = ctx.enter_context(tc.tile_pool(name="outp", bufs=6))
            rcpool = ctx.enter_context(tc.tile_pool(name="rcp", bufs=3))
            pps = ctx.enter_context(
                tc.tile_pool(name="pps", bufs=4, space="PSUM"))
            nps = ctx.enter_context(
                tc.tile_pool(name="nps", bufs=4, space="PSUM"))

            def load_xt(ci):
                tok0 = ci * CHUNK
                xt = []
                for kt in range(NKT):
                    t = xtpool.tile([P, CHUNK], BF16, tag="xt", name="xt")
                    nc.sync.dma_start(
                        t[:], xt_d[kt * P:(kt + 1) * P, tok0:tok0 + CHUNK])
                    xt.append(t)
                return xt

            # queue the first chunk's X^T ahead of everything on sync
            xt_next = load_xt(0)

            # ---- PE warm-up: ramp the clock while DMAs fill ----
            wus = const.tile([P, P], BF16, tag="wus")
            wum = const.tile([P, CHUNK], BF16, tag="wum")
            nc.vector.memset(wus[:], 0.0)
            nc.vector.memset(wum[:], 0.0)
            wup = nps.tile([P, CHUNK], F32, tag="nps", name="wup")
            for _ in range(8):
                nc.tensor.matmul(wup[:], wus[:], wum[:], start=True, stop=True)

            # ---- constants / weights (one-time, gpsimd queue) ----
            w_r = {}
            for nm, drt in (("k", wk_d), ("v", wv_d), ("q", wq_d)):
                w_r[nm] = wpool.tile([P, NKT * CG], BF16, tag=f"w{nm}",
                                     name=f"w{nm}")
            for kt in range(NKT):  # wk tiles first: first matmul needs them
                nc.gpsimd.dma_start(w_r["k"][:, kt * CG:(kt + 1) * CG],
                                    wk_d[kt * P:(kt + 1) * P, :])
            bkr = const.tile([P, CG], BF16, tag="bkr")
            nc.gpsimd.dma_start(bkr[:], bkr_d[:])
            for kt in range(NKT):
                nc.gpsimd.dma_start(w_r["v"][:, kt * CG:(kt + 1) * CG],
                                    wv_d[kt * P:(kt + 1) * P, :])
            bq_sb = const.tile([P, NCT], F32, tag="bqsb")
            nc.gpsimd.dma_start(bq_sb[:], bq_d.rearrange("(c p) -> p c", p=P))
            for kt in range(NKT):
                nc.gpsimd.dma_start(w_r["q"][:, kt * CG:(kt + 1) * CG],
                                    wq_d[kt * P:(kt + 1) * P, :])
            bvbd = const.tile([P, CG], F32, tag="bvbd")
            nc.gpsimd.dma_start(bvbd[:], bvbd_d[:])

            # block-diag ones mask (for the ksum broadcast stationary)
            bd1 = const.tile([P, P], F32, tag="bd1")
            nc.vector.memset(bd1[:], 0.0)
            nc.vector.memset(bd1[0:HD, 0:HD], 1.0)
            nc.vector.memset(bd1[HD:P, HD:P], 1.0)

            # kv accumulator (SBUF f32): per pair [kvA | kvB | ksum | pad]
            kv_sb = wpool.tile([P, NPAIR * PW], F32, tag="kvsb")
            nc.vector.memset(kv_sb[:], 0.0)
            kvb = [wpool.tile([P, P], BF16, tag=f"kvb{i}", name=f"kvb{i}")
                   for i in range(NCT)]
            ksbc = [wpool.tile([P, P], BF16, tag=f"ksbc{i}", name=f"ksbc{i}")
                    for i in range(NCT)]

            kf_c = {}   # chunk -> list of kf tiles (per sub)
            vp_c = {}
            qft_c = {}  # chunk -> list of q_feat^T tiles (per ct)

            def r1_chunk(ci, xt):
                """k/v projections + feature maps for chunk ci."""
                kfs, vps = [], []
                for sub in range(NSUB):
                    psk = pps.tile([P, CG], F32, tag="pps", name="psk")
                    psv = pps.tile([P, CG], F32, tag="pps", name="psv")
                    for kt in range(NKT):
                        nc.tensor.matmul(
                            psk[:], xt[kt][:, sub * P:(sub + 1) * P],
                            w_r["k"][:, kt * CG:(kt + 1) * CG],
                            start=(kt == 0), stop=(kt == NKT - 1))
                    for kt in range(NKT):
                        nc.tensor.matmul(
                            psv[:], xt[kt][:, sub * P:(sub + 1) * P],
                            w_r["v"][:, kt * CG:(kt + 1) * CG],
                            start=(kt == 0), stop=(kt == NKT - 1))
                    # kf = min(exp(x),1) + max(x,0),  x = k + bk
                    x = tmp.tile([P, CG], BF16, tag="t", name="t_x")
                    nc.vector.tensor_tensor(x[:], psk[:], bkr[:], Alu.add)
                    e = tmp.tile([P, CG], BF16, tag="t", name="t_e")
                    nc.scalar.activation(e[:], x[:], Act.Exp)
                    r = tmp.tile([P, CG], BF16, tag="t", name="t_r")
                    nc.vector.tensor_scalar(r[:], x[:], 0.0, None, Alu.max)
                    kf = kfpool.tile([P, CG], BF16, tag="kf", name="kf")
                    nc.vector.scalar_tensor_tensor(
                        kf[:], e[:], 1.0, r[:], Alu.min, Alu.add)
                    kfs.append(kf)
                    # V' = [vA | vB | 1 | 1] per pair (bias folded later)
                    vp = vppool.tile([P, NPAIR * PW], BF16, tag="vp",
                                     name="vp")
                    nc.scalar.activation(
                        vp[:].rearrange("p (r c) -> p r c", c=PW)[:, :, :P],
                        psv[:].rearrange("p (r c) -> p r c", c=P),
                        Act.Copy)
                    nc.vector.memset(
                        vp[:].rearrange("p (r c) -> p r c", c=PW)[:, :, P:],
                        1.0)
                    vps.append(vp)
                kf_c[ci] = kfs
                vp_c[ci] = vps

            def r1_kv(ci):
                """paired kv/ksum accumulation for chunk ci (bf16 matmuls)."""
                kfs, vps = kf_c.pop(ci), vp_c.pop(ci)
                for pr in range(NPAIR):
                    kvt = nps.tile([P, CG], F32, tag="nps", name="kvt")
                    for sub in range(NSUB):
                        nc.tensor.matmul(
                            kvt[:, :PW],
                            kfs[sub][:, pr * P:(pr + 1) * P],
                            vps[sub][:, pr * PW:(pr + 1) * PW],
                            start=(sub == 0), stop=(sub == NSUB - 1))
                    acc = kv_sb[:, pr * PW:(pr + 1) * PW]
                    nc.vector.tensor_tensor(acc, acc, kvt[:, :PW], Alu.add)

            def build_kvb():
                """block-diag bf16 kv (with bv rank-1 fix) + ksum bcast."""
                for pr in range(NPAIR):
                    c0 = pr * PW
                    ks_col = kv_sb[:, c0 + 2 * HD:c0 + 2 * HD + 1]
                    kstg = stg.tile([P, P], F32, tag="kstg", name="kstg")
                    nc.vector.memset(kstg[:], 0.0)
                    nc.vector.tensor_copy(
                        kstg[0:HD, 0:HD], kv_sb[0:HD, c0:c0 + HD])
                    nc.vector.tensor_copy(
                        kstg[HD:P, HD:P], kv_sb[HD:P, c0 + HD:c0 + 2 * HD])
                    # kvb = bvbd * ksum + kv   (rank-1 bv correction)
                    nc.vector.scalar_tensor_tensor(
                        kvb[pr][:], bvbd[:, pr * P:(pr + 1) * P], ks_col,
                        kstg[:], Alu.mult, Alu.add)
                    # ksbc[d,e] = ksum[d] * same_head(d,e)
                    nc.vector.tensor_scalar(
                        ksbc[pr][:], bd1[:], ks_col, None, Alu.mult)

            def r2_chunk(cj, xt):
                """q^T projection + feature map for chunk cj."""
                qft = []
                for ct in range(NCT):
                    ps = pps.tile([P, CHUNK], F32, tag="pps", name="psq")
                    for kt in range(NKT):
                        nc.tensor.matmul(
                            ps[:],
                            w_r["q"][:, kt * CG + ct * P:
                                     kt * CG + (ct + 1) * P],
                            xt[kt][:],
                            start=(kt == 0), stop=(kt == NKT - 1))
                    # qf = min(exp(x),1) + max(x,0),  x = q + bq
                    # exp and relu both read PSUM independently on ACT (no
                    # same-engine producer/consumer ack stall); min+add fuse
                    # into one all-bf16 DVE scalar_tensor_tensor.
                    e = tmp.tile([P, CHUNK], BF16, tag="t", name="t_qe")
                    nc.scalar.activation(e[:], ps[:], Act.Exp,
                                         bias=bq_sb[:, ct:ct + 1])
                    r = tmp.tile([P, CHUNK], BF16, tag="t", name="t_qr")
                    nc.scalar.activation(r[:], ps[:], Act.Relu,
                                         bias=bq_sb[:, ct:ct + 1])
                    qf = qftpool.tile([P, CHUNK], BF16, tag="qft", name="qft")
                    nc.vector.scalar_tensor_tensor(
                        qf[:], e[:], 1.0, r[:], Alu.min, Alu.add)
                    qft.append(qf)
                qft_c[cj] = qft

            def r2_num(cj):
                """transposed numerator / denominator + divide + store."""
                tok0 = cj * CHUNK
                qft = qft_c.pop(cj)
                for ct in range(NCT):
                    dn = nps.tile([P, CHUNK], F32, tag="nps", name="dn")
                    nc.tensor.matmul(dn[:], ksbc[ct][:], qft[ct][:],
                                     start=True, stop=True)
                    # DVE reads at most one PSUM operand per op: reciprocal
                    # evicts 1/den to SBUF, then the multiply reads pn from
                    # PSUM and writes the out tile directly.
                    rc = rcpool.tile([P, CHUNK], F32, tag="rc", name="t_rc")
                    nc.vector.reciprocal(rc[:], dn[:])
                    pn = nps.tile([P, CHUNK], F32, tag="nps", name="pn")
                    nc.tensor.matmul(pn[:], kvb[ct][:], qft[ct][:],
                                     start=True, stop=True)
                    o = outpool.tile([P, CHUNK], F32, tag="out", name="osb")
                    nc.vector.tensor_tensor(o[:], pn[:], rc[:], Alu.mult)
                    nc.gpsimd.dma_start(
                        out_d[ct * P:(ct + 1) * P, tok0:tok0 + CHUNK], o[:])

            # ---- round 1: k/v over all chunks (kv one chunk behind) ----
            for ci in range(NCHUNK):
                xt = xt_next
                xt_next = load_xt((ci + 1) % NCHUNK)  # ci==7 preloads r2 c0
                r1_chunk(ci, xt)
                if ci >= 1:
                    r1_kv(ci - 1)
            r1_kv(NCHUNK - 1)
            build_kvb()

            # ---- round 2: q/num (num one chunk behind the projection) ----
            for cj in range(NCHUNK):
                xt = xt_next
                if cj + 1 < NCHUNK:
                    xt_next = load_xt(cj + 1)
                if cj >= 1:
                    r2_num(cj - 1)
                r2_chunk(cj, xt)
            r2_num(NCHUNK - 1)

    nc.compile()
    return nc


def _get_nc():
    global _CACHED_NC
    if _CACHED_NC is None:
        _CACHED_NC = _build()
    return _CACHED_NC


def _make_in_maps(hidden_states, Wq, bq, Wk, bk, Wv, bv):
    import ml_dtypes
    bf16 = ml_dtypes.bfloat16

    hs = np.asarray(hidden_states, np.float32)
    wq = np.asarray(Wq, np.float32)
    wk = np.asarray(Wk, np.float32)
    wv = np.asarray(Wv, np.float32)
    bqf = np.asarray(bq, np.float32)
    bkf = np.asarray(bk, np.float32)
    bvf = np.asarray(bv, np.float32)

    xbfs = [np.ascontiguousarray(hs[b].T).astype(bf16) for b in range(B)]
    in_maps = []
    for c in range(NCORES):
        b, g = divmod(c, 2)
        sl = slice(g * CG, (g + 1) * CG)
        bv_c = bvf[sl]
        # block-diag bv: rows 0:64 -> even head of pair, 64:128 -> odd head
        bvbd = np.zeros((P, CG), np.float32)
        for pr in range(NPAIR):
            bvbd[0:HD, pr * P:pr * P + HD] = bv_c[pr * P:pr * P + HD]
            bvbd[HD:P, pr * P + HD:(pr + 1) * P] = bv_c[pr * P + HD:
                                                        (pr + 1) * P]
        in_maps.append({
            "xt": xbfs[b],
            "wk": np.ascontiguousarray(wk[:, sl]).astype(bf16),
            "wv": np.ascontiguousarray(wv[:, sl]).astype(bf16),
            "wq": np.ascontiguousarray(wq[:, sl]).astype(bf16),
            "bq": np.ascontiguousarray(bqf[sl]),
            "bkr": np.ascontiguousarray(
                np.broadcast_to(bkf[sl][None, :], (P, CG))).astype(bf16),
            "bvbd": bvbd,
        })
    return in_maps


def _run(in_maps, **kwargs):
    from concourse.bass_utils import run_bass_kernel_spmd
    nc = _get_nc()
    return run_bass_kernel_spmd(nc, in_maps, core_ids=list(range(NCORES)),
                                **kwargs)


def _assemble(results):
    out = np.empty((B, S, D), np.float32)
    for c in range(NCORES):
        b, g = divmod(c, 2)
        out[b, :, g * CG:(g + 1) * CG] = results[c]["out"].T
    return out


def kernel(hidden_states, Wq, bq, Wk, bk, Wv, bv):
    in_maps = _make_in_maps(hidden_states, Wq, bq, Wk, bk, Wv, bv)
    res = _run(in_maps)
    return _assemble(res.results)


# revision 17
# speedup vs baseline: 1.1628x; 1.1628x over previous
"""Trainium2 Bass kernel for BertLinearSelfAttention (linear attention).

Reference computation (per batch b, head h):
    q,k,v = X @ W{q,k,v} + b{q,k,v}            # [S, D] -> heads of 64
    qf, kf = elu(q)+1, elu(k)+1                # = min(exp(x),1) + max(x,0)
    kv[d,e]  = sum_s kf[s,d] v[s,e]            # [64, 64]
    ksum[d]  = sum_s kf[s,d]
    out[s,e] = (sum_d qf[s,d] kv[d,e]) / (sum_d qf[s,d] ksum[d])

Sharding: 8 cores = (4 batches) x (2 head-groups of 8 heads / 512 proj cols).

Everything runs in bf16 (f32 PSUM accumulation): bf16 is the fastest
matmul dtype at ANY output size (fp32r drops to 1/4 rate below 256
moving rows), stationary loads are half-cost, input DMA halves, and DVE
elementwise ops on all-SBUF bf16 operands run at up to 4x.  Measured
end-to-end L2 error of the full bf16 pipeline is ~2e-3 (tolerance 2e-2).

Round 1 (k/v) per 512-token chunk: X^T tiles stationary, W moving
(N=512).  Feature map kf = exp(min(x,0)) + max(x,0):
    x  = k + bk          (DVE, psum + broadcast-bias tile)
    rel= Relu(-x)        (ACT, scale=-1)
    m  = Exp(-rel)       (ACT, scale=-1)  == min(exp(x),1)
    r  = max(x,0)        (DVE 4x)
    kf = m + r           (DVE 4x, bf16)
kv/ksum matmuls run with heads PAIRED: moving layout [vA|vB|ones|pad]
(N=130) against a [128tok,128] kf stationary -> block-diag kv plus a
shared ksum column, accumulated per chunk into SBUF f32.

Round 2 (q/out): q^T projection (Wq stationary, X^T moving, N=512
tokens).  The numerator is computed TRANSPOSED: the bf16 block-diag kv
is the stationary operand, qf^T the moving one -> one 512-row matmul
per head-pair.  The per-head denominator is produced directly in
broadcast form by a second matmul whose stationary is the block-diag
ksum replication [d,e]=ksum[d]*same_head(d,e); out = num/den is then a
single DVE divide per column tile.  Output is stored transposed
([CG, S]) and transposed back on the host.

bv is never added to v on-device: kv_true = kv_nobias + ksum (x) bv is
applied once to the tiny kv matrix (bv enters kv linearly).

The PE is warmed up with dummy matmuls during the initial DMA fill (the
clock p-state needs ~3us of continuous work to reach 2.4 GHz), and all
cross-engine consumers are emitted one chunk behind their producers so
the PE never waits on the ACT/DVE feature chain.
"""

import os
import sys

import numpy as np

_REPO = "/opt/trn_rl_repo"
if os.path.isdir(_REPO) and _REPO not in sys.path:
    sys.path.insert(0, _REPO)

B, S, D, H, HD = 4, 4096, 1024, 16, 64
NCORES = 8
CG = 512            # projection columns per core (8 heads)
NH = CG // HD       # 8 heads per core
NPAIR = NH // 2     # 4 head pairs (= column tiles of 128)
PW = 130            # pair width in kv psum: [vA(64) | vB(64) | ones | pad]
CHUNK = 512         # tokens per chunk
NSUB = CHUNK // 128     # 4 token sub-tiles per chunk
NCHUNK = S // CHUNK     # 8 chunks
NKT = D // 128          # 8 contraction tiles
NCT = CG // 128         # 4 column tiles (= head pairs)
P = 128

_CACHED_NC = None


def _build():
    import concourse.tile as tile
    from concourse import bacc, mybir
    from contextlib import ExitStack

    F32 = mybir.dt.float32
    BF16 = mybir.dt.bfloat16
    Alu = mybir.AluOpType
    Act = mybir.ActivationFunctionType

    nc = bacc.Bacc("TRN2", target_bir_lowering=False, debug=False,
                   num_devices=NCORES)

    xt_d = nc.dram_tensor("xt", [D, S], BF16, kind="ExternalInput").ap()
    wk_d = nc.dram_tensor("wk", [D, CG], BF16, kind="ExternalInput").ap()
    wv_d = nc.dram_tensor("wv", [D, CG], BF16, kind="ExternalInput").ap()
    wq_d = nc.dram_tensor("wq", [D, CG], BF16, kind="ExternalInput").ap()
    bq_d = nc.dram_tensor("bq", [CG], F32, kind="ExternalInput").ap()
    bkr_d = nc.dram_tensor("bkr", [P, CG], BF16, kind="ExternalInput").ap()
    bvbd_d = nc.dram_tensor("bvbd", [P, CG], F32, kind="ExternalInput").ap()
    out_d = nc.dram_tensor("out", [CG, S], F32, kind="ExternalOutput").ap()

    with tile.TileContext(nc) as tc:
        with ExitStack() as ctx:
            const = ctx.enter_context(tc.tile_pool(name="const", bufs=1))
            wpool = ctx.enter_context(tc.tile_pool(name="wpool", bufs=1))
            xtpool = ctx.enter_context(tc.tile_pool(name="xtpool", bufs=16))
            kfpool = ctx.enter_context(tc.tile_pool(name="kfpool", bufs=9))
            vppool = ctx.enter_context(tc.tile_pool(name="vppool", bufs=9))
            qftpool = ctx.enter_context(tc.tile_pool(name="qftpool", bufs=9))
            tmp = ctx.enter_context(tc.tile_pool(name="tmp", bufs=12))
            stg = ctx.enter_context(tc.tile_pool(name="stg", bufs=2))
            outpool = ctx.enter_context(tc.tile_pool(name="outp", bufs=6))
            rcpool = ctx.enter_context(tc.tile_pool(name="rcp", bufs=3))
            pps = ctx.enter_context(
                tc.tile_pool(name="pps", bufs=4, space="PSUM"))
            nps = ctx.enter_context(
                tc.tile_pool(name="nps", bufs=4, space="PSUM"))

            def load_xt(ci):
                tok0 = ci * CHUNK
                xt = []
                for kt in range(NKT):
                    t = xtpool.tile([P, CHUNK], BF16, tag="xt", name="xt")
                    nc.sync.dma_start(
                        t[:], xt_d[kt * P:(kt + 1) * P, tok0:tok0 + CHUNK])
                    xt.append(t)
                return xt

            # queue the first chunk's X^T ahead of everything on sync
            xt_next = load_xt(0)

            # ---- PE warm-up: ramp the clock while DMAs fill ----
            wus = const.tile([P, P], BF16, tag="wus")
            wum = const.tile([P, CHUNK], BF16, tag="wum")
            nc.vector.memset(wus[:], 0.0)
            nc.vector.memset(wum[:], 0.0)
            wup = nps.tile([P, CHUNK], F32, tag="nps", name="wup")
            for _ in range(8):
                nc.tensor.matmul(wup[:], wus[:], wum[:], start=True, stop=True)

            # ---- constants / weights (one-time, gpsimd queue) ----
            w_r = {}
            for nm, drt in (("k", wk_d), ("v", wv_d), ("q", wq_d)):
                w_r[nm] = wpool.tile([P, NKT * CG], BF16, tag=f"w{nm}",
                                     name=f"w{nm}")
            for kt in range(NKT):  # wk tiles first: first matmul needs them
                nc.gpsimd.dma_start(w_r["k"][:, kt * CG:(kt + 1) * CG],
                                    wk_d[kt * P:(kt + 1) * P, :])
            bkr = const.tile([P, CG], BF16, tag="bkr")
            nc.gpsimd.dma_start(bkr[:], bkr_d[:])
            for kt in range(NKT):
                nc.gpsimd.dma_start(w_r["v"][:, kt * CG:(kt + 1) * CG],
                                    wv_d[kt * P:(kt + 1) * P, :])
            bq_sb = const.tile([P, NCT], F32, tag="bqsb")
            nc.gpsimd.dma_start(bq_sb[:], bq_d.rearrange("(c p) -> p c", p=P))
            for kt in range(NKT):
                nc.gpsimd.dma_start(w_r["q"][:, kt * CG:(kt + 1) * CG],
                                    wq_d[kt * P:(kt + 1) * P, :])
            bvbd = const.tile([P, CG], F32, tag="bvbd")
            nc.gpsimd.dma_start(bvbd[:], bvbd_d[:])

            # block-diag ones mask (for the ksum broadcast stationary)
            bd1 = const.tile([P, P], F32, tag="bd1")
            nc.vector.memset(bd1[:], 0.0)
            nc.vector.memset(bd1[0:HD, 0:HD], 1.0)
            nc.vector.memset(bd1[HD:P, HD:P], 1.0)

            # kv accumulator (SBUF f32): per pair [kvA | kvB | ksum | pad]
            kv_sb = wpool.tile([P, NPAIR * PW], F32, tag="kvsb")
            nc.vector.memset(kv_sb[:], 0.0)
            kvb = [wpool.tile([P, P], BF16, tag=f"kvb{i}", name=f"kvb{i}")
                   for i in range(NCT)]
            ksbc = [wpool.tile([P, P], BF16, tag=f"ksbc{i}", name=f"ksbc{i}")
                    for i in range(NCT)]

            kf_c = {}   # chunk -> list of kf tiles (per sub)
            vp_c = {}
            qft_c = {}  # chunk -> list of q_feat^T tiles (per ct)

            def r1_chunk(ci, xt):
                """k/v projections + feature maps for chunk ci."""
                kfs, vps = [], []
                for sub in range(NSUB):
                    psk = pps.tile([P, CG], F32, tag="pps", name="psk")
                    psv = pps.tile([P, CG], F32, tag="pps", name="psv")
                    for kt in range(NKT):
                        nc.tensor.matmul(
                            psk[:], xt[kt][:, sub * P:(sub + 1) * P],
                            w_r["k"][:, kt * CG:(kt + 1) * CG],
                            start=(kt == 0), stop=(kt == NKT - 1))
                    for kt in range(NKT):
                        nc.tensor.matmul(
                            psv[:], xt[kt][:, sub * P:(sub + 1) * P],
                            w_r["v"][:, kt * CG:(kt + 1) * CG],
                            start=(kt == 0), stop=(kt == NKT - 1))
                    # kf = min(exp(x),1) + max(x,0),  x = k + bk
                    x = tmp.tile([P, CG], BF16, tag="t", name="t_x")
                    nc.vector.tensor_tensor(x[:], psk[:], bkr[:], Alu.add)
                    e = tmp.tile([P, CG], BF16, tag="t", name="t_e")
                    nc.scalar.activation(e[:], x[:], Act.Exp)
                    r = tmp.tile([P, CG], BF16, tag="t", name="t_r")
                    nc.vector.tensor_scalar(r[:], x[:], 0.0, None, Alu.max)
                    # NOTE: scalar_tensor_tensor with bf16 inputs is ~6x slow
                    # on HW; two plain bf16 ops run at the fast 2x rate.
                    m = tmp.tile([P, CG], BF16, tag="t", name="t_m")
                    nc.vector.tensor_scalar(m[:], e[:], 1.0, None, Alu.min)
                    kf = kfpool.tile([P, CG], BF16, tag="kf", name="kf")
                    nc.vector.tensor_tensor(kf[:], m[:], r[:], Alu.add)
                    kfs.append(kf)
                    # V' = [vA | vB | 1 | 1] per pair (bias folded later)
                    vp = vppool.tile([P, NPAIR * PW], BF16, tag="vp",
                                     name="vp")
                    nc.scalar.activation(
                        vp[:].rearrange("p (r c) -> p r c", c=PW)[:, :, :P],
                        psv[:].rearrange("p (r c) -> p r c", c=P),
                        Act.Copy)
                    nc.vector.memset(
                        vp[:].rearrange("p (r c) -> p r c", c=PW)[:, :, P:],
                        1.0)
                    vps.append(vp)
                kf_c[ci] = kfs
                vp_c[ci] = vps

            def r1_kv(ci):
                """paired kv/ksum accumulation for chunk ci (bf16 matmuls)."""
                kfs, vps = kf_c.pop(ci), vp_c.pop(ci)
                for pr in range(NPAIR):
                    kvt = nps.tile([P, CG], F32, tag="nps", name="kvt")
                    for sub in range(NSUB):
                        nc.tensor.matmul(
                            kvt[:, :PW],
                            kfs[sub][:, pr * P:(pr + 1) * P],
                            vps[sub][:, pr * PW:(pr + 1) * PW],
                            start=(sub == 0), stop=(sub == NSUB - 1))
                    acc = kv_sb[:, pr * PW:(pr + 1) * PW]
                    nc.vector.tensor_tensor(acc, acc, kvt[:, :PW], Alu.add)

            def build_kvb():
                """block-diag bf16 kv (with bv rank-1 fix) + ksum bcast."""
                for pr in range(NPAIR):
                    c0 = pr * PW
                    ks_col = kv_sb[:, c0 + 2 * HD:c0 + 2 * HD + 1]
                    kstg = stg.tile([P, P], F32, tag="kstg", name="kstg")
                    nc.vector.memset(kstg[:], 0.0)
                    nc.vector.tensor_copy(
                        kstg[0:HD, 0:HD], kv_sb[0:HD, c0:c0 + HD])
                    nc.vector.tensor_copy(
                        kstg[HD:P, HD:P], kv_sb[HD:P, c0 + HD:c0 + 2 * HD])
                    # kvb = bvbd * ksum + kv   (rank-1 bv correction)
                    nc.vector.scalar_tensor_tensor(
                        kvb[pr][:], bvbd[:, pr * P:(pr + 1) * P], ks_col,
                        kstg[:], Alu.mult, Alu.add)
                    # ksbc[d,e] = ksum[d] * same_head(d,e)
                    nc.vector.tensor_scalar(
                        ksbc[pr][:], bd1[:], ks_col, None, Alu.mult)

            def r2_chunk(cj, xt):
                """q^T projection + feature map for chunk cj."""
                qft = []
                for ct in range(NCT):
                    ps = pps.tile([P, CHUNK], F32, tag="pps", name="psq")
                    for kt in range(NKT):
                        nc.tensor.matmul(
                            ps[:],
                            w_r["q"][:, kt * CG + ct * P:
                                     kt * CG + (ct + 1) * P],
                            xt[kt][:],
                            start=(kt == 0), stop=(kt == NKT - 1))
                    # qf = min(exp(x),1) + max(x,0),  x = q + bq
                    # exp and relu both read PSUM independently on ACT (no
                    # same-engine producer/consumer ack stall); min+add fuse
                    # into one all-bf16 DVE scalar_tensor_tensor.
                    e = tmp.tile([P, CHUNK], BF16, tag="t", name="t_qe")
                    nc.scalar.activation(e[:], ps[:], Act.Exp,
                                         bias=bq_sb[:, ct:ct + 1])
                    r = tmp.tile([P, CHUNK], BF16, tag="t", name="t_qr")
                    nc.scalar.activation(r[:], ps[:], Act.Relu,
                                         bias=bq_sb[:, ct:ct + 1])
                    m = tmp.tile([P, CHUNK], BF16, tag="t", name="t_qm")
                    nc.vector.tensor_scalar(m[:], e[:], 1.0, None, Alu.min)
                    qf = qftpool.tile([P, CHUNK], BF16, tag="qft", name="qft")
                    nc.vector.tensor_tensor(qf[:], m[:], r[:], Alu.add)
                    qft.append(qf)
                qft_c[cj] = qft

            def r2_num(cj):
                """transposed numerator / denominator + divide + store."""
                tok0 = cj * CHUNK
                qft = qft_c.pop(cj)
                for ct in range(NCT):
                    dn = nps.tile([P, CHUNK], F32, tag="nps", name="dn")
                    nc.tensor.matmul(dn[:], ksbc[ct][:], qft[ct][:],
                                     start=True, stop=True)
                    # DVE reads at most one PSUM operand per op: reciprocal
                    # evicts 1/den to SBUF, then the multiply reads pn from
                    # PSUM and writes the out tile directly.
                    rc = rcpool.tile([P, CHUNK], F32, tag="rc", name="t_rc")
                    nc.vector.reciprocal(rc[:], dn[:])
                    pn = nps.tile([P, CHUNK], F32, tag="nps", name="pn")
                    nc.tensor.matmul(pn[:], kvb[ct][:], qft[ct][:],
                                     start=True, stop=True)
                    o = outpool.tile([P, CHUNK], F32, tag="out", name="osb")
                    nc.vector.tensor_tensor(o[:], pn[:], rc[:], Alu.mult)
                    nc.gpsimd.dma_start(
                        out_d[ct * P:(ct + 1) * P, tok0:tok0 + CHUNK], o[:])

            # ---- round 1: k/v over all chunks (kv one chunk behind) ----
            for ci in range(NCHUNK):
                xt = xt_next
                xt_next = load_xt((ci + 1) % NCHUNK)  # ci==7 preloads r2 c0
                r1_chunk(ci, xt)
                if ci >= 1:
                    r1_kv(ci - 1)
            r1_kv(NCHUNK - 1)
            build_kvb()

            # ---- round 2: q/num (num one chunk behind the projection) ----
            for cj in range(NCHUNK):
                xt = xt_next
                if cj + 1 < NCHUNK:
                    xt_next = load_xt(cj + 1)
                if cj >= 1:
                    r2_num(cj - 1)
                r2_chunk(cj, xt)
            r2_num(NCHUNK - 1)

    nc.compile()
    return nc


def _get_nc():
    global _CACHED_NC
    if _CACHED_NC is None:
        _CACHED_NC = _build()
    return _CACHED_NC


def _make_in_maps(hidden_states, Wq, bq, Wk, bk, Wv, bv):
    import ml_dtypes
    bf16 = ml_dtypes.bfloat16

    hs = np.asarray(hidden_states, np.float32)
    wq = np.asarray(Wq, np.float32)
    wk = np.asarray(Wk, np.float32)
    wv = np.asarray(Wv, np.float32)
    bqf = np.asarray(bq, np.float32)
    bkf = np.asarray(bk, np.float32)
    bvf = np.asarray(bv, np.float32)

    xbfs = [np.ascontiguousarray(hs[b].T).astype(bf16) for b in range(B)]
    in_maps = []
    for c in range(NCORES):
        b, g = divmod(c, 2)
        sl = slice(g * CG, (g + 1) * CG)
        bv_c = bvf[sl]
        # block-diag bv: rows 0:64 -> even head of pair, 64:128 -> odd head
        bvbd = np.zeros((P, CG), np.float32)
        for pr in range(NPAIR):
            bvbd[0:HD, pr * P:pr * P + HD] = bv_c[pr * P:pr * P + HD]
            bvbd[HD:P, pr * P + HD:(pr + 1) * P] = bv_c[pr * P + HD:
                                                        (pr + 1) * P]
        in_maps.append({
            "xt": xbfs[b],
            "wk": np.ascontiguousarray(wk[:, sl]).astype(bf16),
            "wv": np.ascontiguousarray(wv[:, sl]).astype(bf16),
            "wq": np.ascontiguousarray(wq[:, sl]).astype(bf16),
            "bq": np.ascontiguousarray(bqf[sl]),
            "bkr": np.ascontiguousarray(
                np.broadcast_to(bkf[sl][None, :], (P, CG))).astype(bf16),
            "bvbd": bvbd,
        })
    return in_maps


def _run(in_maps, **kwargs):
    from concourse.bass_utils import run_bass_kernel_spmd
    nc = _get_nc()
    return run_bass_kernel_spmd(nc, in_maps, core_ids=list(range(NCORES)),
                                **kwargs)


def _assemble(results):
    out = np.empty((B, S, D), np.float32)
    for c in range(NCORES):
        b, g = divmod(c, 2)
        out[b, :, g * CG:(g + 1) * CG] = results[c]["out"].T
    return out


def kernel(hidden_states, Wq, bq, Wk, bk, Wv, bv):
    in_maps = _make_in_maps(hidden_states, Wq, bq, Wk, bk, Wv, bv)
    res = _run(in_maps)
    return _assemble(res.results)


# revision 18
# speedup vs baseline: 1.5927x; 1.3697x over previous
"""Trainium2 Bass kernel for BertLinearSelfAttention (linear attention).

Reference computation (per batch b, head h):
    q,k,v = X @ W{q,k,v} + b{q,k,v}            # [S, D] -> heads of 64
    qf, kf = elu(q)+1, elu(k)+1                # = min(exp(x),1) + max(x,0)
    kv[d,e]  = sum_s kf[s,d] v[s,e]            # [64, 64]
    ksum[d]  = sum_s kf[s,d]
    out[s,e] = (sum_d qf[s,d] kv[d,e]) / (sum_d qf[s,d] ksum[d])

Sharding: 8 cores = (4 batches) x (2 head-groups of 8 heads / 512 proj cols).

Everything runs in bf16 (f32 PSUM accumulation): bf16 is the fastest
matmul dtype at ANY output size (fp32r drops to 1/4 rate below 256
moving rows), stationary loads are half-cost, input DMA halves, and DVE
elementwise ops on all-SBUF bf16 operands run at up to 4x.  Measured
end-to-end L2 error of the full bf16 pipeline is ~2e-3 (tolerance 2e-2).

Round 1 (k/v) per 512-token chunk: X^T tiles stationary, W moving
(N=512).  Feature map kf = exp(min(x,0)) + max(x,0):
    x  = k + bk          (DVE, psum + broadcast-bias tile)
    rel= Relu(-x)        (ACT, scale=-1)
    m  = Exp(-rel)       (ACT, scale=-1)  == min(exp(x),1)
    r  = max(x,0)        (DVE 4x)
    kf = m + r           (DVE 4x, bf16)
kv/ksum matmuls run with heads PAIRED: moving layout [vA|vB|ones|pad]
(N=130) against a [128tok,128] kf stationary -> block-diag kv plus a
shared ksum column, accumulated per chunk into SBUF f32.

Round 2 (q/out): q^T projection (Wq stationary, X^T moving, N=512
tokens).  The numerator is computed TRANSPOSED: the bf16 block-diag kv
is the stationary operand, qf^T the moving one -> one 512-row matmul
per head-pair.  The per-head denominator is produced directly in
broadcast form by a second matmul whose stationary is the block-diag
ksum replication [d,e]=ksum[d]*same_head(d,e); out = num/den is then a
single DVE divide per column tile.  Output is stored transposed
([CG, S]) and transposed back on the host.

bv is never added to v on-device: kv_true = kv_nobias + ksum (x) bv is
applied once to the tiny kv matrix (bv enters kv linearly).

The PE is warmed up with dummy matmuls during the initial DMA fill (the
clock p-state needs ~3us of continuous work to reach 2.4 GHz), and all
cross-engine consumers are emitted one chunk behind their producers so
the PE never waits on the ACT/DVE feature chain.
"""

import os
import sys

import numpy as np

_REPO = "/opt/trn_rl_repo"
if os.path.isdir(_REPO) and _REPO not in sys.path:
    sys.path.insert(0, _REPO)

B, S, D, H, HD = 4, 4096, 1024, 16, 64
NCORES = 8
CG = 512            # projection columns per core (8 heads)
NH = CG // HD       # 8 heads per core
NPAIR = NH // 2     # 4 head pairs (= column tiles of 128)
PW = 130            # pair width in kv psum: [vA(64) | vB(64) | ones | pad]
CHUNK = 512         # tokens per chunk
NSUB = CHUNK // 128     # 4 token sub-tiles per chunk
NCHUNK = S // CHUNK     # 8 chunks
NKT = D // 128          # 8 contraction tiles
NCT = CG // 128         # 4 column tiles (= head pairs)
P = 128

_CACHED_NC = None


def _build():
    import concourse.tile as tile
    from concourse import bacc, mybir
    from contextlib import ExitStack

    F32 = mybir.dt.float32
    BF16 = mybir.dt.bfloat16
    Alu = mybir.AluOpType
    Act = mybir.ActivationFunctionType

    nc = bacc.Bacc("TRN2", target_bir_lowering=False, debug=False,
                   num_devices=NCORES)

    xt_d = nc.dram_tensor("xt", [D, S], BF16, kind="ExternalInput").ap()
    wk_d = nc.dram_tensor("wk", [D, CG], BF16, kind="ExternalInput").ap()
    wv_d = nc.dram_tensor("wv", [D, CG], BF16, kind="ExternalInput").ap()
    wq_d = nc.dram_tensor("wq", [D, CG], BF16, kind="ExternalInput").ap()
    bq_d = nc.dram_tensor("bq", [CG], F32, kind="ExternalInput").ap()
    bkr_d = nc.dram_tensor("bkr", [P, CG], BF16, kind="ExternalInput").ap()
    bvbd_d = nc.dram_tensor("bvbd", [P, CG], F32, kind="ExternalInput").ap()
    out_d = nc.dram_tensor("out", [CG, S], F32, kind="ExternalOutput").ap()

    with tile.TileContext(nc) as tc:
        with ExitStack() as ctx:
            const = ctx.enter_context(tc.tile_pool(name="const", bufs=1))
            wpool = ctx.enter_context(tc.tile_pool(name="wpool", bufs=1))
            xtpool = ctx.enter_context(tc.tile_pool(name="xtpool", bufs=16))
            kfpool = ctx.enter_context(tc.tile_pool(name="kfpool", bufs=9))
            vppool = ctx.enter_context(tc.tile_pool(name="vppool", bufs=9))
            qftpool = ctx.enter_context(tc.tile_pool(name="qftpool", bufs=9))
            tmp = ctx.enter_context(tc.tile_pool(name="tmp", bufs=12))
            stg = ctx.enter_context(tc.tile_pool(name="stg", bufs=2))
            outpool = ctx.enter_context(tc.tile_pool(name="outp", bufs=6))
            rcpool = ctx.enter_context(tc.tile_pool(name="rcp", bufs=3))
            pps = ctx.enter_context(
                tc.tile_pool(name="pps", bufs=4, space="PSUM"))
            nps = ctx.enter_context(
                tc.tile_pool(name="nps", bufs=4, space="PSUM"))

            def load_xt(ci):
                tok0 = ci * CHUNK
                xt = []
                for kt in range(NKT):
                    t = xtpool.tile([P, CHUNK], BF16, tag="xt", name="xt")
                    nc.sync.dma_start(
                        t[:], xt_d[kt * P:(kt + 1) * P, tok0:tok0 + CHUNK])
                    xt.append(t)
                return xt

            # queue the first chunk's X^T ahead of everything on sync
            xt_next = load_xt(0)

            # ---- PE warm-up: ramp the clock while DMAs fill ----
            wus = const.tile([P, P], BF16, tag="wus")
            wum = const.tile([P, CHUNK], BF16, tag="wum")
            nc.vector.memset(wus[:], 0.0)
            nc.vector.memset(wum[:], 0.0)
            wup = nps.tile([P, CHUNK], F32, tag="nps", name="wup")
            for _ in range(8):
                nc.tensor.matmul(wup[:], wus[:], wum[:], start=True, stop=True)

            # ---- constants / weights (one-time, gpsimd queue) ----
            w_r = {}
            for nm, drt in (("k", wk_d), ("v", wv_d), ("q", wq_d)):
                w_r[nm] = wpool.tile([P, NKT * CG], BF16, tag=f"w{nm}",
                                     name=f"w{nm}")
            for kt in range(NKT):  # wk tiles first: first matmul needs them
                nc.gpsimd.dma_start(w_r["k"][:, kt * CG:(kt + 1) * CG],
                                    wk_d[kt * P:(kt + 1) * P, :])
            bkr = const.tile([P, CG], BF16, tag="bkr")
            nc.gpsimd.dma_start(bkr[:], bkr_d[:])
            for kt in range(NKT):
                nc.gpsimd.dma_start(w_r["v"][:, kt * CG:(kt + 1) * CG],
                                    wv_d[kt * P:(kt + 1) * P, :])
            bq_sb = const.tile([P, NCT], F32, tag="bqsb")
            nc.gpsimd.dma_start(bq_sb[:], bq_d.rearrange("(c p) -> p c", p=P))
            for kt in range(NKT):
                nc.gpsimd.dma_start(w_r["q"][:, kt * CG:(kt + 1) * CG],
                                    wq_d[kt * P:(kt + 1) * P, :])
            bvbd = const.tile([P, CG], F32, tag="bvbd")
            nc.gpsimd.dma_start(bvbd[:], bvbd_d[:])

            # block-diag ones mask (for the ksum broadcast stationary)
            bd1 = const.tile([P, P], F32, tag="bd1")
            nc.vector.memset(bd1[:], 0.0)
            nc.vector.memset(bd1[0:HD, 0:HD], 1.0)
            nc.vector.memset(bd1[HD:P, HD:P], 1.0)

            # kv accumulator (SBUF f32): per pair [kvA | kvB | ksum | pad]
            kv_sb = wpool.tile([P, NPAIR * PW], F32, tag="kvsb")
            nc.vector.memset(kv_sb[:], 0.0)
            kvb = [wpool.tile([P, P], BF16, tag=f"kvb{i}", name=f"kvb{i}")
                   for i in range(NCT)]
            ksbc = [wpool.tile([P, P], BF16, tag=f"ksbc{i}", name=f"ksbc{i}")
                    for i in range(NCT)]

            kf_c = {}   # chunk -> list of kf tiles (per sub)
            vp_c = {}
            qft_c = {}  # chunk -> list of q_feat^T tiles (per ct)

            def r1_chunk(ci, xt):
                """k/v projections + feature maps for chunk ci."""
                kfs, vps = [], []
                for sub in range(NSUB):
                    psk = pps.tile([P, CG], F32, tag="pps", name="psk")
                    psv = pps.tile([P, CG], F32, tag="pps", name="psv")
                    for kt in range(NKT):
                        nc.tensor.matmul(
                            psk[:], xt[kt][:, sub * P:(sub + 1) * P],
                            w_r["k"][:, kt * CG:(kt + 1) * CG],
                            start=(kt == 0), stop=(kt == NKT - 1))
                    for kt in range(NKT):
                        nc.tensor.matmul(
                            psv[:], xt[kt][:, sub * P:(sub + 1) * P],
                            w_r["v"][:, kt * CG:(kt + 1) * CG],
                            start=(kt == 0), stop=(kt == NKT - 1))
                    # kf = min(exp(x),1) + max(x,0),  x = k + bk
                    x = tmp.tile([P, CG], BF16, tag="t", name="t_x")
                    nc.vector.tensor_tensor(x[:], psk[:], bkr[:], Alu.add)
                    e = tmp.tile([P, CG], BF16, tag="t", name="t_e")
                    nc.scalar.activation(e[:], x[:], Act.Exp)
                    r = tmp.tile([P, CG], BF16, tag="t", name="t_r")
                    nc.vector.tensor_scalar(r[:], x[:], 0.0, None, Alu.max)
                    # NOTE: scalar_tensor_tensor with bf16 inputs is ~6x slow
                    # on HW; two plain bf16 ops run at the fast 2x rate.
                    m = tmp.tile([P, CG], BF16, tag="t", name="t_m")
                    nc.vector.tensor_scalar(m[:], e[:], 1.0, None, Alu.min)
                    kf = kfpool.tile([P, CG], BF16, tag="kf", name="kf")
                    nc.vector.tensor_tensor(kf[:], m[:], r[:], Alu.add)
                    kfs.append(kf)
                    # V' = [vA | vB | 1 | 1] per pair (bias folded later)
                    vp = vppool.tile([P, NPAIR * PW], BF16, tag="vp",
                                     name="vp")
                    nc.scalar.activation(
                        vp[:].rearrange("p (r c) -> p r c", c=PW)[:, :, :P],
                        psv[:].rearrange("p (r c) -> p r c", c=P),
                        Act.Copy)
                    nc.vector.memset(
                        vp[:].rearrange("p (r c) -> p r c", c=PW)[:, :, P:],
                        1.0)
                    vps.append(vp)
                kf_c[ci] = kfs
                vp_c[ci] = vps

            def r1_kv(ci):
                """paired kv/ksum accumulation for chunk ci (bf16 matmuls)."""
                kfs, vps = kf_c.pop(ci), vp_c.pop(ci)
                for pr in range(NPAIR):
                    kvt = nps.tile([P, CG], F32, tag="nps", name="kvt")
                    for sub in range(NSUB):
                        nc.tensor.matmul(
                            kvt[:, :PW],
                            kfs[sub][:, pr * P:(pr + 1) * P],
                            vps[sub][:, pr * PW:(pr + 1) * PW],
                            start=(sub == 0), stop=(sub == NSUB - 1))
                    acc = kv_sb[:, pr * PW:(pr + 1) * PW]
                    nc.vector.tensor_tensor(acc, acc, kvt[:, :PW], Alu.add)

            def build_kvb():
                """block-diag bf16 kv (with bv rank-1 fix) + ksum bcast."""
                for pr in range(NPAIR):
                    c0 = pr * PW
                    ks_col = kv_sb[:, c0 + 2 * HD:c0 + 2 * HD + 1]
                    kstg = stg.tile([P, P], F32, tag="kstg", name="kstg")
                    nc.vector.memset(kstg[:], 0.0)
                    nc.vector.tensor_copy(
                        kstg[0:HD, 0:HD], kv_sb[0:HD, c0:c0 + HD])
                    nc.vector.tensor_copy(
                        kstg[HD:P, HD:P], kv_sb[HD:P, c0 + HD:c0 + 2 * HD])
                    # kvb = bvbd * ksum + kv   (rank-1 bv correction)
                    nc.vector.scalar_tensor_tensor(
                        kvb[pr][:], bvbd[:, pr * P:(pr + 1) * P], ks_col,
                        kstg[:], Alu.mult, Alu.add)
                    # ksbc[d,e] = ksum[d] * same_head(d,e)
                    nc.vector.tensor_scalar(
                        ksbc[pr][:], bd1[:], ks_col, None, Alu.mult)

            def r2_chunk(cj, xt):
                """q^T projection + feature map for chunk cj."""
                qft = []
                for ct in range(NCT):
                    ps = pps.tile([P, CHUNK], F32, tag="pps", name="psq")
                    for kt in range(NKT):
                        nc.tensor.matmul(
                            ps[:],
                            w_r["q"][:, kt * CG + ct * P:
                                     kt * CG + (ct + 1) * P],
                            xt[kt][:],
                            start=(kt == 0), stop=(kt == NKT - 1))
                    # qf = min(exp(x),1) + max(x,0),  x = q + bq
                    # exp and relu both read PSUM independently on ACT (no
                    # same-engine producer/consumer ack stall); min+add fuse
                    # into one all-bf16 DVE scalar_tensor_tensor.
                    e = tmp.tile([P, CHUNK], BF16, tag="t", name="t_qe")
                    nc.scalar.activation(e[:], ps[:], Act.Exp,
                                         bias=bq_sb[:, ct:ct + 1])
                    r = tmp.tile([P, CHUNK], BF16, tag="t", name="t_qr")
                    nc.scalar.activation(r[:], ps[:], Act.Relu,
                                         bias=bq_sb[:, ct:ct + 1])
                    m = tmp.tile([P, CHUNK], BF16, tag="t", name="t_qm")
                    nc.vector.tensor_scalar(m[:], e[:], 1.0, None, Alu.min)
                    qf = qftpool.tile([P, CHUNK], BF16, tag="qft", name="qft")
                    nc.vector.tensor_tensor(qf[:], m[:], r[:], Alu.add)
                    qft.append(qf)
                qft_c[cj] = qft

            def r2_num(cj):
                """transposed numerator / denominator + divide + store."""
                tok0 = cj * CHUNK
                qft = qft_c.pop(cj)
                for ct in range(NCT):
                    dn = nps.tile([P, CHUNK], F32, tag="nps", name="dn")
                    nc.tensor.matmul(dn[:], ksbc[ct][:], qft[ct][:],
                                     start=True, stop=True)
                    # DVE reads at most one PSUM operand per op: reciprocal
                    # evicts 1/den to SBUF, then the multiply reads pn from
                    # PSUM and writes the out tile directly.  The full
                    # `reciprocal` is a multi-pass iteration (~3.6us per
                    # [128,512] tile on HW); the ~18-bit single-op approx is
                    # 5x faster and far more accurate than needed (den is a
                    # well-conditioned positive sum, tolerance is 2e-2).
                    rc = rcpool.tile([P, CHUNK], F32, tag="rc", name="t_rc")
                    nc.vector.reciprocal_approx_fast(out=rc[:], in_=dn[:])
                    pn = nps.tile([P, CHUNK], F32, tag="nps", name="pn")
                    nc.tensor.matmul(pn[:], kvb[ct][:], qft[ct][:],
                                     start=True, stop=True)
                    o = outpool.tile([P, CHUNK], F32, tag="out", name="osb")
                    nc.vector.tensor_tensor(o[:], pn[:], rc[:], Alu.mult)
                    nc.gpsimd.dma_start(
                        out_d[ct * P:(ct + 1) * P, tok0:tok0 + CHUNK], o[:])

            # ---- round 1: k/v over all chunks (kv one chunk behind) ----
            for ci in range(NCHUNK):
                xt = xt_next
                xt_next = load_xt((ci + 1) % NCHUNK)  # ci==7 preloads r2 c0
                r1_chunk(ci, xt)
                if ci >= 1:
                    r1_kv(ci - 1)
            r1_kv(NCHUNK - 1)
            build_kvb()

            # ---- round 2: q/num (num one chunk behind the projection) ----
            for cj in range(NCHUNK):
                xt = xt_next
                if cj + 1 < NCHUNK:
                    xt_next = load_xt(cj + 1)
                if cj >= 1:
                    r2_num(cj - 1)
                r2_chunk(cj, xt)
            r2_num(NCHUNK - 1)

    nc.compile()
    return nc


def _get_nc():
    global _CACHED_NC
    if _CACHED_NC is None:
        _CACHED_NC = _build()
    return _CACHED_NC


def _make_in_maps(hidden_states, Wq, bq, Wk, bk, Wv, bv):
    import ml_dtypes
    bf16 = ml_dtypes.bfloat16

    hs = np.asarray(hidden_states, np.float32)
    wq = np.asarray(Wq, np.float32)
    wk = np.asarray(Wk, np.float32)
    wv = np.asarray(Wv, np.float32)
    bqf = np.asarray(bq, np.float32)
    bkf = np.asarray(bk, np.float32)
    bvf = np.asarray(bv, np.float32)

    xbfs = [np.ascontiguousarray(hs[b].T).astype(bf16) for b in range(B)]
    in_maps = []
    for c in range(NCORES):
        b, g = divmod(c, 2)
        sl = slice(g * CG, (g + 1) * CG)
        bv_c = bvf[sl]
        # block-diag bv: rows 0:64 -> even head of pair, 64:128 -> odd head
        bvbd = np.zeros((P, CG), np.float32)
        for pr in range(NPAIR):
            bvbd[0:HD, pr * P:pr * P + HD] = bv_c[pr * P:pr * P + HD]
            bvbd[HD:P, pr * P + HD:(pr + 1) * P] = bv_c[pr * P + HD:
                                                        (pr + 1) * P]
        in_maps.append({
            "xt": xbfs[b],
            "wk": np.ascontiguousarray(wk[:, sl]).astype(bf16),
            "wv": np.ascontiguousarray(wv[:, sl]).astype(bf16),
            "wq": np.ascontiguousarray(wq[:, sl]).astype(bf16),
            "bq": np.ascontiguousarray(bqf[sl]),
            "bkr": np.ascontiguousarray(
                np.broadcast_to(bkf[sl][None, :], (P, CG))).astype(bf16),
            "bvbd": bvbd,
        })
    return in_maps


def _run(in_maps, **kwargs):
    from concourse.bass_utils import run_bass_kernel_spmd
    nc = _get_nc()
    return run_bass_kernel_spmd(nc, in_maps, core_ids=list(range(NCORES)),
                                **kwargs)


def _assemble(results):
    out = np.empty((B, S, D), np.float32)
    for c in range(NCORES):
        b, g = divmod(c, 2)
        out[b, :, g * CG:(g + 1) * CG] = results[c]["out"].T
    return out


def kernel(hidden_states, Wq, bq, Wk, bk, Wv, bv):
    in_maps = _make_in_maps(hidden_states, Wq, bq, Wk, bk, Wv, bv)
    res = _run(in_maps)
    return _assemble(res.results)


# revision 24
# speedup vs baseline: 1.5946x; 1.0012x over previous
"""Trainium2 Bass kernel for BertLinearSelfAttention (linear attention).

Reference computation (per batch b, head h):
    q,k,v = X @ W{q,k,v} + b{q,k,v}            # [S, D] -> heads of 64
    qf, kf = elu(q)+1, elu(k)+1                # = min(exp(x),1) + max(x,0)
    kv[d,e]  = sum_s kf[s,d] v[s,e]            # [64, 64]
    ksum[d]  = sum_s kf[s,d]
    out[s,e] = (sum_d qf[s,d] kv[d,e]) / (sum_d qf[s,d] ksum[d])

Sharding: 8 cores = (4 batches) x (2 head-groups of 8 heads / 512 proj cols).

Everything runs in bf16 (f32 PSUM accumulation): bf16 is the fastest
matmul dtype at ANY output size (fp32r drops to 1/4 rate below 256
moving rows), stationary loads are half-cost, input DMA halves, and DVE
elementwise ops on all-SBUF bf16 operands run at up to 4x.  Measured
end-to-end L2 error of the full bf16 pipeline is ~2e-3 (tolerance 2e-2).

Round 1 (k/v) per 512-token chunk: X^T tiles stationary, W moving
(N=512).  Feature map kf = exp(min(x,0)) + max(x,0):
    x  = k + bk          (DVE, psum + broadcast-bias tile)
    rel= Relu(-x)        (ACT, scale=-1)
    m  = Exp(-rel)       (ACT, scale=-1)  == min(exp(x),1)
    r  = max(x,0)        (DVE 4x)
    kf = m + r           (DVE 4x, bf16)
kv/ksum matmuls run with heads PAIRED: moving layout [vA|vB|ones|pad]
(N=130) against a [128tok,128] kf stationary -> block-diag kv plus a
shared ksum column, accumulated per chunk into SBUF f32.

Round 2 (q/out): q^T projection (Wq stationary, X^T moving, N=512
tokens).  The numerator is computed TRANSPOSED: the bf16 block-diag kv
is the stationary operand, qf^T the moving one -> one 512-row matmul
per head-pair.  The per-head denominator is produced directly in
broadcast form by a second matmul whose stationary is the block-diag
ksum replication [d,e]=ksum[d]*same_head(d,e); out = num/den is then a
single DVE divide per column tile.  Output is stored transposed
([CG, S]) and transposed back on the host.

bv is never added to v on-device: kv_true = kv_nobias + ksum (x) bv is
applied once to the tiny kv matrix (bv enters kv linearly).

The PE is warmed up with dummy matmuls during the initial DMA fill (the
clock p-state needs ~3us of continuous work to reach 2.4 GHz), and all
cross-engine consumers are emitted one chunk behind their producers so
the PE never waits on the ACT/DVE feature chain.
"""

import os
import sys

import numpy as np

_REPO = "/opt/trn_rl_repo"
if os.path.isdir(_REPO) and _REPO not in sys.path:
    sys.path.insert(0, _REPO)

B, S, D, H, HD = 4, 4096, 1024, 16, 64
NCORES = 8
CG = 512            # projection columns per core (8 heads)
NH = CG // HD       # 8 heads per core
NPAIR = NH // 2     # 4 head pairs (= column tiles of 128)
PW = 130            # pair width in kv psum: [vA(64) | vB(64) | ones | pad]
CHUNK = 512         # tokens per chunk
NSUB = CHUNK // 128     # 4 token sub-tiles per chunk
NCHUNK = S // CHUNK     # 8 chunks
NKT = D // 128          # 8 contraction tiles
NCT = CG // 128         # 4 column tiles (= head pairs)
P = 128

_CACHED_NC = None


def _build():
    import concourse.tile as tile
    from concourse import bacc, mybir
    from contextlib import ExitStack

    F32 = mybir.dt.float32
    BF16 = mybir.dt.bfloat16
    Alu = mybir.AluOpType
    Act = mybir.ActivationFunctionType

    nc = bacc.Bacc("TRN2", target_bir_lowering=False, debug=False,
                   num_devices=NCORES)

    xt_d = nc.dram_tensor("xt", [D, S], BF16, kind="ExternalInput").ap()
    wk_d = nc.dram_tensor("wk", [D, CG], BF16, kind="ExternalInput").ap()
    wv_d = nc.dram_tensor("wv", [D, CG], BF16, kind="ExternalInput").ap()
    wq_d = nc.dram_tensor("wq", [D, CG], BF16, kind="ExternalInput").ap()
    bq_d = nc.dram_tensor("bq", [CG], F32, kind="ExternalInput").ap()
    bkr_d = nc.dram_tensor("bkr", [P, CG], BF16, kind="ExternalInput").ap()
    bvbd_d = nc.dram_tensor("bvbd", [P, CG], F32, kind="ExternalInput").ap()
    out_d = nc.dram_tensor("out", [CG, S], F32, kind="ExternalOutput").ap()

    with tile.TileContext(nc) as tc:
        with ExitStack() as ctx:
            const = ctx.enter_context(tc.tile_pool(name="const", bufs=1))
            wpool = ctx.enter_context(tc.tile_pool(name="wpool", bufs=1))
            xtpool = ctx.enter_context(tc.tile_pool(name="xtpool", bufs=16))
            kfpool = ctx.enter_context(tc.tile_pool(name="kfpool", bufs=9))
            vppool = ctx.enter_context(tc.tile_pool(name="vppool", bufs=9))
            qftpool = ctx.enter_context(tc.tile_pool(name="qftpool", bufs=9))
            tmp = ctx.enter_context(tc.tile_pool(name="tmp", bufs=12))
            stg = ctx.enter_context(tc.tile_pool(name="stg", bufs=2))
            outpool = ctx.enter_context(tc.tile_pool(name="outp", bufs=6))
            rcpool = ctx.enter_context(tc.tile_pool(name="rcp", bufs=3))
            pps = ctx.enter_context(
                tc.tile_pool(name="pps", bufs=4, space="PSUM"))
            nps = ctx.enter_context(
                tc.tile_pool(name="nps", bufs=4, space="PSUM"))

            def load_xt(ci):
                tok0 = ci * CHUNK
                xt = []
                for kt in range(NKT):
                    t = xtpool.tile([P, CHUNK], BF16, tag="xt", name="xt")
                    nc.sync.dma_start(
                        t[:], xt_d[kt * P:(kt + 1) * P, tok0:tok0 + CHUNK])
                    xt.append(t)
                return xt

            # queue the first chunk's X^T ahead of everything on sync
            xt_next = load_xt(0)

            # ---- PE warm-up: ramp the clock while DMAs fill ----
            wus = const.tile([P, P], BF16, tag="wus")
            wum = const.tile([P, CHUNK], BF16, tag="wum")
            nc.vector.memset(wus[:], 0.0)
            nc.vector.memset(wum[:], 0.0)
            wup = nps.tile([P, CHUNK], F32, tag="nps", name="wup")
            # enough dummy work to keep the PE clock ramping through the
            # whole initial DMA fill (~8us): an idle gap before the first
            # real matmul would reset the p-state to 1.2GHz for 3us.
            for _ in range(18):
                nc.tensor.matmul(wup[:], wus[:], wum[:], start=True, stop=True)

            # ---- constants / weights (one-time, gpsimd queue) ----
            w_r = {}
            for nm, drt in (("k", wk_d), ("v", wv_d), ("q", wq_d)):
                w_r[nm] = wpool.tile([P, NKT * CG], BF16, tag=f"w{nm}",
                                     name=f"w{nm}")
            for kt in range(NKT):  # wk tiles first: first matmul needs them
                nc.gpsimd.dma_start(w_r["k"][:, kt * CG:(kt + 1) * CG],
                                    wk_d[kt * P:(kt + 1) * P, :])
            bkr = const.tile([P, CG], BF16, tag="bkr")
            nc.gpsimd.dma_start(bkr[:], bkr_d[:])
            for kt in range(NKT):
                nc.gpsimd.dma_start(w_r["v"][:, kt * CG:(kt + 1) * CG],
                                    wv_d[kt * P:(kt + 1) * P, :])
            bq_sb = const.tile([P, NCT], F32, tag="bqsb")
            nc.gpsimd.dma_start(bq_sb[:], bq_d.rearrange("(c p) -> p c", p=P))
            for kt in range(NKT):
                nc.gpsimd.dma_start(w_r["q"][:, kt * CG:(kt + 1) * CG],
                                    wq_d[kt * P:(kt + 1) * P, :])
            bvbd = const.tile([P, CG], F32, tag="bvbd")
            nc.gpsimd.dma_start(bvbd[:], bvbd_d[:])

            # block-diag ones mask (for the ksum broadcast stationary)
            bd1 = const.tile([P, P], F32, tag="bd1")
            nc.vector.memset(bd1[:], 0.0)
            nc.vector.memset(bd1[0:HD, 0:HD], 1.0)
            nc.vector.memset(bd1[HD:P, HD:P], 1.0)

            # kv accumulator (SBUF f32): per pair [kvA | kvB | ksum | pad]
            kv_sb = wpool.tile([P, NPAIR * PW], F32, tag="kvsb")
            nc.vector.memset(kv_sb[:], 0.0)
            kvb = [wpool.tile([P, P], BF16, tag=f"kvb{i}", name=f"kvb{i}")
                   for i in range(NCT)]
            ksbc = [wpool.tile([P, P], BF16, tag=f"ksbc{i}", name=f"ksbc{i}")
                    for i in range(NCT)]

            kf_c = {}   # chunk -> list of kf tiles (per sub)
            vp_c = {}
            qft_c = {}  # chunk -> list of q_feat^T tiles (per ct)

            def r1_chunk(ci, xt):
                """k/v projections + feature maps for chunk ci."""
                kfs, vps = [], []
                for sub in range(NSUB):
                    psk = pps.tile([P, CG], F32, tag="pps", name="psk")
                    psv = pps.tile([P, CG], F32, tag="pps", name="psv")
                    for kt in range(NKT):
                        nc.tensor.matmul(
                            psk[:], xt[kt][:, sub * P:(sub + 1) * P],
                            w_r["k"][:, kt * CG:(kt + 1) * CG],
                            start=(kt == 0), stop=(kt == NKT - 1))
                    for kt in range(NKT):
                        nc.tensor.matmul(
                            psv[:], xt[kt][:, sub * P:(sub + 1) * P],
                            w_r["v"][:, kt * CG:(kt + 1) * CG],
                            start=(kt == 0), stop=(kt == NKT - 1))
                    # kf = min(exp(x),1) + max(x,0),  x = k + bk
                    x = tmp.tile([P, CG], BF16, tag="t", name="t_x")
                    nc.vector.tensor_tensor(x[:], psk[:], bkr[:], Alu.add)
                    e = tmp.tile([P, CG], BF16, tag="t", name="t_e")
                    nc.scalar.activation(e[:], x[:], Act.Exp)
                    r = tmp.tile([P, CG], BF16, tag="t", name="t_r")
                    nc.vector.tensor_scalar(r[:], x[:], 0.0, None, Alu.max)
                    # NOTE: scalar_tensor_tensor with bf16 inputs is ~6x slow
                    # on HW; two plain bf16 ops run at the fast 2x rate.
                    m = tmp.tile([P, CG], BF16, tag="t", name="t_m")
                    nc.vector.tensor_scalar(m[:], e[:], 1.0, None, Alu.min)
                    kf = kfpool.tile([P, CG], BF16, tag="kf", name="kf")
                    nc.vector.tensor_tensor(kf[:], m[:], r[:], Alu.add)
                    kfs.append(kf)
                    # V' = [vA | vB | 1 | 1] per pair (bias folded later)
                    vp = vppool.tile([P, NPAIR * PW], BF16, tag="vp",
                                     name="vp")
                    nc.scalar.activation(
                        vp[:].rearrange("p (r c) -> p r c", c=PW)[:, :, :P],
                        psv[:].rearrange("p (r c) -> p r c", c=P),
                        Act.Copy)
                    nc.vector.memset(
                        vp[:].rearrange("p (r c) -> p r c", c=PW)[:, :, P:],
                        1.0)
                    vps.append(vp)
                kf_c[ci] = kfs
                vp_c[ci] = vps

            def r1_kv(ci):
                """paired kv/ksum accumulation for chunk ci (bf16 matmuls)."""
                kfs, vps = kf_c.pop(ci), vp_c.pop(ci)
                for pr in range(NPAIR):
                    kvt = nps.tile([P, CG], F32, tag="nps", name="kvt")
                    for sub in range(NSUB):
                        nc.tensor.matmul(
                            kvt[:, :PW],
                            kfs[sub][:, pr * P:(pr + 1) * P],
                            vps[sub][:, pr * PW:(pr + 1) * PW],
                            start=(sub == 0), stop=(sub == NSUB - 1))
                    acc = kv_sb[:, pr * PW:(pr + 1) * PW]
                    nc.vector.tensor_tensor(acc, acc, kvt[:, :PW], Alu.add)

            def build_kvb():
                """block-diag bf16 kv (with bv rank-1 fix) + ksum bcast."""
                for pr in range(NPAIR):
                    c0 = pr * PW
                    ks_col = kv_sb[:, c0 + 2 * HD:c0 + 2 * HD + 1]
                    kstg = stg.tile([P, P], F32, tag="kstg", name="kstg")
                    nc.vector.memset(kstg[:], 0.0)
                    nc.vector.tensor_copy(
                        kstg[0:HD, 0:HD], kv_sb[0:HD, c0:c0 + HD])
                    nc.vector.tensor_copy(
                        kstg[HD:P, HD:P], kv_sb[HD:P, c0 + HD:c0 + 2 * HD])
                    # kvb = bvbd * ksum + kv   (rank-1 bv correction)
                    nc.vector.scalar_tensor_tensor(
                        kvb[pr][:], bvbd[:, pr * P:(pr + 1) * P], ks_col,
                        kstg[:], Alu.mult, Alu.add)
                    # ksbc[d,e] = ksum[d] * same_head(d,e)
                    nc.vector.tensor_scalar(
                        ksbc[pr][:], bd1[:], ks_col, None, Alu.mult)

            def r2_chunk(cj, xt, eager=False):
                """q^T projection + feature map for chunk cj.

                eager=True (last chunk): interleave each ct's den/num chain
                two projection groups later, so the final out chain overlaps
                the remaining projections instead of trailing serially.
                """
                qft = []
                for ct in range(NCT):
                    ps = pps.tile([P, CHUNK], F32, tag="pps", name="psq")
                    for kt in range(NKT):
                        nc.tensor.matmul(
                            ps[:],
                            w_r["q"][:, kt * CG + ct * P:
                                     kt * CG + (ct + 1) * P],
                            xt[kt][:],
                            start=(kt == 0), stop=(kt == NKT - 1))
                    # qf = min(exp(x),1) + max(x,0),  x = q + bq
                    # exp and relu both read PSUM independently on ACT (no
                    # same-engine producer/consumer ack stall); min+add fuse
                    # into one all-bf16 DVE scalar_tensor_tensor.
                    e = tmp.tile([P, CHUNK], BF16, tag="t", name="t_qe")
                    nc.scalar.activation(e[:], ps[:], Act.Exp,
                                         bias=bq_sb[:, ct:ct + 1])
                    r = tmp.tile([P, CHUNK], BF16, tag="t", name="t_qr")
                    nc.scalar.activation(r[:], ps[:], Act.Relu,
                                         bias=bq_sb[:, ct:ct + 1])
                    m = tmp.tile([P, CHUNK], BF16, tag="t", name="t_qm")
                    nc.vector.tensor_scalar(m[:], e[:], 1.0, None, Alu.min)
                    qf = qftpool.tile([P, CHUNK], BF16, tag="qft", name="qft")
                    nc.vector.tensor_tensor(qf[:], m[:], r[:], Alu.add)
                    qft.append(qf)
                    if eager and ct >= 2:
                        _num_ct(cj, ct - 2, qft[ct - 2])
                qft_c[cj] = qft

            def _num_ct(cj, ct, qf):
                """den/num matmuls + 1/den + scale + store for one ct."""
                tok0 = cj * CHUNK
                dn = nps.tile([P, CHUNK], F32, tag="nps", name="dn")
                nc.tensor.matmul(dn[:], ksbc[ct][:], qf[:],
                                 start=True, stop=True)
                # DVE reads at most one PSUM operand per op: reciprocal
                # evicts 1/den to SBUF, then the multiply reads pn from
                # PSUM and writes the out tile directly.  The full
                # `reciprocal` is a multi-pass iteration (~3.6us per
                # [128,512] tile on HW); the ~18-bit single-op approx is
                # 5x faster and far more accurate than needed (den is a
                # well-conditioned positive sum, tolerance is 2e-2).
                rc = rcpool.tile([P, CHUNK], F32, tag="rc", name="t_rc")
                nc.vector.reciprocal_approx_fast(out=rc[:], in_=dn[:])
                pn = nps.tile([P, CHUNK], F32, tag="nps", name="pn")
                nc.tensor.matmul(pn[:], kvb[ct][:], qf[:],
                                 start=True, stop=True)
                o = outpool.tile([P, CHUNK], F32, tag="out", name="osb")
                nc.vector.tensor_tensor(o[:], pn[:], rc[:], Alu.mult)
                nc.gpsimd.dma_start(
                    out_d[ct * P:(ct + 1) * P, tok0:tok0 + CHUNK], o[:])

            def r2_num(cj):
                """den/num + divide + store for all cts of chunk cj."""
                qft = qft_c.pop(cj)
                for ct in range(NCT):
                    _num_ct(cj, ct, qft[ct])

            # ---- round 1: k/v over all chunks (kv one chunk behind) ----
            for ci in range(NCHUNK):
                xt = xt_next
                xt_next = load_xt((ci + 1) % NCHUNK)  # ci==7 preloads r2 c0
                r1_chunk(ci, xt)
                if ci >= 1:
                    r1_kv(ci - 1)
            r1_kv(NCHUNK - 1)
            build_kvb()

            # ---- round 2: q/num (num one chunk behind the projection) ----
            for cj in range(NCHUNK):
                xt = xt_next
                if cj + 1 < NCHUNK:
                    xt_next = load_xt(cj + 1)
                if cj >= 1:
                    r2_num(cj - 1)
                r2_chunk(cj, xt, eager=(cj == NCHUNK - 1))
            # eager mode already emitted cts 0..1; finish the last two
            qft = qft_c.pop(NCHUNK - 1)
            for ct in (2, 3):
                _num_ct(NCHUNK - 1, ct, qft[ct])

    nc.compile()
    return nc


def _get_nc():
    global _CACHED_NC
    if _CACHED_NC is None:
        _CACHED_NC = _build()
    return _CACHED_NC


def _make_in_maps(hidden_states, Wq, bq, Wk, bk, Wv, bv):
    import ml_dtypes
    bf16 = ml_dtypes.bfloat16

    hs = np.asarray(hidden_states, np.float32)
    wq = np.asarray(Wq, np.float32)
    wk = np.asarray(Wk, np.float32)
    wv = np.asarray(Wv, np.float32)
    bqf = np.asarray(bq, np.float32)
    bkf = np.asarray(bk, np.float32)
    bvf = np.asarray(bv, np.float32)

    xbfs = [np.ascontiguousarray(hs[b].T).astype(bf16) for b in range(B)]
    in_maps = []
    for c in range(NCORES):
        b, g = divmod(c, 2)
        sl = slice(g * CG, (g + 1) * CG)
        bv_c = bvf[sl]
        # block-diag bv: rows 0:64 -> even head of pair, 64:128 -> odd head
        bvbd = np.zeros((P, CG), np.float32)
        for pr in range(NPAIR):
            bvbd[0:HD, pr * P:pr * P + HD] = bv_c[pr * P:pr * P + HD]
            bvbd[HD:P, pr * P + HD:(pr + 1) * P] = bv_c[pr * P + HD:
                                                        (pr + 1) * P]
        in_maps.append({
            "xt": xbfs[b],
            "wk": np.ascontiguousarray(wk[:, sl]).astype(bf16),
            "wv": np.ascontiguousarray(wv[:, sl]).astype(bf16),
            "wq": np.ascontiguousarray(wq[:, sl]).astype(bf16),
            "bq": np.ascontiguousarray(bqf[sl]),
            "bkr": np.ascontiguousarray(
                np.broadcast_to(bkf[sl][None, :], (P, CG))).astype(bf16),
            "bvbd": bvbd,
        })
    return in_maps


def _run(in_maps, **kwargs):
    from concourse.bass_utils import run_bass_kernel_spmd
    nc = _get_nc()
    return run_bass_kernel_spmd(nc, in_maps, core_ids=list(range(NCORES)),
                                **kwargs)


def _assemble(results):
    out = np.empty((B, S, D), np.float32)
    for c in range(NCORES):
        b, g = divmod(c, 2)
        out[b, :, g * CG:(g + 1) * CG] = results[c]["out"].T
    return out


def kernel(hidden_states, Wq, bq, Wk, bk, Wv, bv):
    in_maps = _make_in_maps(hidden_states, Wq, bq, Wk, bk, Wv, bv)
    res = _run(in_maps)
    return _assemble(res.results)
